# revision 4
# baseline (speedup 1.0000x reference)
"""BoostedCausalAttention on 8 trn2 NeuronCores.

Sharding: core c -> (batch b = c//4, head-group g = c%4, 4 heads each).
Within a 4-core batch group (Megatron-style):
  - qkv projections + attention computed per head-group in "transposed"
    layout (feature on partitions, token on free axis), fp32r matmuls.
  - AllGather of pred^T (round 0) for the residual; AllToAll of pred^T /
    corr^T column-shards so each core gets the full-channel slice for its
    512-token strip of the gate MLP + output projection (static offsets).
  - Final output: each core emits y[512 tokens, 1024] (pre-bias); host
    concatenates and adds bo.
"""

from contextlib import ExitStack

import numpy as np

import concourse.bass as bass
import concourse.bacc as bacc
import concourse.tile as tile
import concourse.mybir as mybir
from concourse import bass_utils

B, T, D = 2, 2048, 1024
H, DH = 16, 64
SCALE = DH ** -0.5
G = 4            # head groups (cores per batch)
HG = H // G      # heads per core = 4
CP = HG * DH     # channels per core = 256
TS = T // G      # token slice per core for gate/output phase = 512
KC = D // 128    # contraction chunks over D = 8
MASK_VAL = -30000.0

F32 = mybir.dt.float32
F32R = mybir.dt.float32r
ID = mybir.ActivationFunctionType.Identity
EXP = mybir.ActivationFunctionType.Exp
SIG = mybir.ActivationFunctionType.Sigmoid

GROUPS = [[0, 1, 2, 3], [4, 5, 6, 7]]


def _build():
    nc = bacc.Bacc("TRN2", target_bir_lowering=False, debug=False, num_devices=8)

    xT = nc.dram_tensor("xT", [D, T], F32R, kind="ExternalInput")
    wqk0 = nc.dram_tensor("wqk0", [D, 2 * CP], F32R, kind="ExternalInput")
    wv0 = nc.dram_tensor("wv0", [D, CP], F32R, kind="ExternalInput")
    bqk0 = nc.dram_tensor("bqk0", [128, 4], F32, kind="ExternalInput")
    bv0 = nc.dram_tensor("bv0", [64, HG], F32, kind="ExternalInput")
    wqk1 = nc.dram_tensor("wqk1", [D, 2 * CP], F32R, kind="ExternalInput")
    wv1 = nc.dram_tensor("wv1", [D, CP], F32R, kind="ExternalInput")
    bqk1 = nc.dram_tensor("bqk1", [128, 4], F32, kind="ExternalInput")
    bv1 = nc.dram_tensor("bv1", [64, HG], F32, kind="ExternalInput")
    wg = nc.dram_tensor("wg", [2 * D, D], F32R, kind="ExternalInput")
    bg = nc.dram_tensor("bg", [128, D // 128], F32, kind="ExternalInput")
    wo = nc.dram_tensor("wo", [D, D], F32R, kind="ExternalInput")
    mask = nc.dram_tensor("mask", [128, 128], F32, kind="ExternalInput")
    onesc = nc.dram_tensor("onesc", [128, HG], F32R, kind="ExternalInput")
    y = nc.dram_tensor("y", [TS, D], F32, kind="ExternalOutput")

    with tile.TileContext(nc) as tc, ExitStack() as glb:
        consts = glb.enter_context(tc.tile_pool(name="consts", bufs=1))
        # 8KB-per-partition slots: x^T/residual tiles, later reused by Wg
        big8 = glb.enter_context(tc.tile_pool(name="big8", bufs=8))
        dpool = glb.enter_context(tc.tile_pool(name="dpool", bufs=1, space="DRAM"))
        dsmall = glb.enter_context(tc.tile_pool(name="dsmall", bufs=4, space="DRAM"))

        # ---- constants ----
        bqk_sb = [consts.tile([128, 4], F32, name=f"bqk_sb{r}") for r in range(2)]
        bv_sb = [consts.tile([64, HG], F32, name=f"bv_sb{r}") for r in range(2)]
        bg_sb = consts.tile([128, D // 128], F32)
        mask_sb = consts.tile([128, 128], F32)
        nc.sync.dma_start(bqk_sb[0][:], bqk0[:, :])
        nc.sync.dma_start(bqk_sb[1][:], bqk1[:, :])
        nc.sync.dma_start(bv_sb[0][:], bv0[:, :])
        nc.sync.dma_start(bv_sb[1][:], bv1[:, :])
        nc.sync.dma_start(bg_sb[:], bg[:, :])
        nc.sync.dma_start(mask_sb[:], mask[:, :])

        # ---- x^T resident (becomes residual in place after round 0) ----
        xt = []
        for kc in range(KC):
            t_ = big8.tile([128, T], F32R, name=f"xt{kc}", tag="b8")
            nc.sync.dma_start(t_[:], xT[128 * kc:128 * (kc + 1), :])
            xt.append(t_)

        # ---- collective DRAM buffers ----
        pred_part = dpool.tile([CP, T], F32R)          # round-0 AG input
        pred_full = dpool.tile([D, T], F32R)           # round-0 AG output
        corr_part = dpool.tile([CP, T], F32R)          # round-1 AG input
        corr_full = dpool.tile([D, T], F32R)           # round-1 AG output

        with ExitStack() as att:
            wqk_pool = att.enter_context(tc.tile_pool(name="wqk", bufs=KC))
            wv_pool = att.enter_context(tc.tile_pool(name="wv", bufs=KC))
            qk_pool = att.enter_context(tc.tile_pool(name="qk", bufs=4))
            vaug_pool = att.enter_context(tc.tile_pool(name="vaug", bufs=16))
            p_pool = att.enter_context(tc.tile_pool(name="pp", bufs=3))
            o_pool = att.enter_context(tc.tile_pool(name="op", bufs=4))
            sm_pool = att.enter_context(tc.tile_pool(name="sm", bufs=2))
            ld_pool = att.enter_context(tc.tile_pool(name="ld", bufs=2))
            psum = att.enter_context(tc.tile_pool(name="psum", bufs=3, space="PSUM"))
            avps = att.enter_context(tc.tile_pool(name="avps", bufs=4, space="PSUM"))

            def proj_qk(wqk_d, src, biasc, rnd):
                """q^T|k^T [512 rows, T] as 4 tiles [128, T] (0-1: q, 2-3: k)."""
                wt = []
                for kc in range(KC):
                    t_ = wqk_pool.tile([128, 2 * CP], F32R,
                                       name=f"wqk{rnd}_{kc}", tag="wqk")
                    nc.sync.dma_start(t_[:], wqk_d[128 * kc:128 * (kc + 1), :])
                    wt.append(t_)
                qk = []
                for jc in range(4):
                    qk.append(qk_pool.tile([128, T], F32R,
                                           name=f"qk{rnd}_{jc}", tag="qk"))
                for jc in range(4):
                    for t4 in range(4):
                        ps = psum.tile([128, 512], F32, tag="ps", name="ps_pqk")
                        for kc in range(KC):
                            nc.tensor.matmul(
                                ps[:], wt[kc][:, 128 * jc:128 * (jc + 1)],
                                src[kc][:, 512 * t4:512 * (t4 + 1)],
                                start=(kc == 0), stop=(kc == KC - 1))
                        nc.scalar.activation(qk[jc][:, 512 * t4:512 * (t4 + 1)],
                                             ps[:], ID, bias=biasc[:, jc:jc + 1])
                return qk

            def proj_v(wv_d, src, rnd):
                """v in natural layout + ones col: 16 tiles [128, HG, DH+1]."""
                wt = []
                for kc in range(KC):
                    t_ = wv_pool.tile([128, CP], F32R,
                                      name=f"wv{rnd}_{kc}", tag="wv")
                    nc.sync.dma_start(t_[:], wv_d[128 * kc:128 * (kc + 1), :])
                    wt.append(t_)
                va = []
                for tb in range(16):
                    t_ = vaug_pool.tile([128, HG, DH + 1], F32R,
                                        name=f"va{rnd}_{tb}", tag="va")
                    ps = psum.tile([128, 512], F32, tag="ps", name="ps_pv")
                    for kc in range(KC):
                        nc.tensor.matmul(
                            ps[:, 0:CP], src[kc][:, 128 * tb:128 * (tb + 1)],
                            wt[kc][:], start=(kc == 0), stop=(kc == KC - 1))
                    nc.scalar.copy(
                        t_[:, :, 0:DH],
                        ps[:, 0:CP].rearrange("p (h d) -> p h d", h=HG))
                    nc.sync.dma_start(t_[:, :, DH:DH + 1], onesc[:, :, None])
                    va.append(t_)
                return va

            def attend(qk, va, biasv, part_dst):
                """Causal attention for 4 heads; writes normalized pred^T
                parts straight to DRAM (AG layout and/or A2A layout)."""
                for hh in range(2):      # head pairs share PE via row groups
                    for q4 in range(4):
                        nblk = 4 * (q4 + 1)
                        av = [avps.tile([DH + 1, 512], F32, tag="av",
                                        name=f"av{h2}") for h2 in range(2)]
                        for kb in range(nblk):
                            for h2 in range(2):
                                h = 2 * hh + h2
                                base = 64 * h2
                                qt, kt = qk[hh], qk[2 + hh]
                                diag = kb - 4 * q4
                                c0 = max(0, 128 * diag)
                                npr = 512 - c0
                                ps = psum.tile([128, 512], F32, tag="ps",
                                               name=f"s{h2}")
                                nc.tensor.matmul(
                                    ps[:, 0:npr],
                                    kt[base:base + 64,
                                       128 * kb:128 * (kb + 1)],
                                    qt[base:base + 64,
                                       512 * q4 + c0:512 * (q4 + 1)],
                                    start=True, stop=True)
                                if diag >= 0:
                                    nc.vector.tensor_add(
                                        ps[:, 0:128], ps[:, 0:128], mask_sb[:])
                                p = p_pool.tile([128, 512], F32R, tag="p",
                                                name=f"p{h2}")
                                nc.scalar.activation(p[:, 0:npr], ps[:, 0:npr],
                                                     EXP, scale=SCALE)
                                nc.tensor.matmul(
                                    av[h2][:, c0:512], va[kb][:, h, :],
                                    p[:, 0:npr],
                                    start=(kb == 0), stop=(kb == nblk - 1))
                        for h2 in range(2):
                            h = 2 * hh + h2
                            recip = sm_pool.tile([1, 512], F32, tag="recip",
                                                 name="recip")
                            nc.vector.reciprocal(recip[:],
                                                 av[h2][DH:DH + 1, :])
                            rd = dsmall.tile([1, 512], F32, tag="rd", name="rd")
                            nc.sync.dma_start(rd[:, :], recip[:])
                            rb = sm_pool.tile([64, 512], F32, tag="rb",
                                              name="rb")
                            nc.sync.dma_start(
                                rb[:], bass.AP(tensor=rd.tensor, offset=0,
                                               ap=[[0, 64], [1, 512]]))
                            o = o_pool.tile([64, 512], F32R, tag="o", name="o")
                            nc.vector.tensor_mul(o[:], av[h2][0:DH, :], rb[:])
                            nc.vector.tensor_scalar_add(o[:], o[:],
                                                        biasv[:, h:h + 1])
                            nc.sync.dma_start(
                                part_dst[64 * h:64 * (h + 1),
                                         512 * q4:512 * (q4 + 1)], o[:])

            # ================= round 0 =================
            qk0 = proj_qk(wqk0, xt, bqk_sb[0], 0)
            va0 = proj_v(wv0, xt, 0)
            attend(qk0, va0, bv_sb[0], pred_part)

            nc.gpsimd.collective_compute(
                "AllGather", mybir.AluOpType.bypass, replica_groups=GROUPS,
                ins=[pred_part[:, :]], outs=[pred_full[:, :]])

            # residual in place: xt <- xt - pred^T
            for kc in range(KC):
                pt = ld_pool.tile([128, T], F32R, tag="predld", name="predld")
                nc.sync.dma_start(pt[:], pred_full[128 * kc:128 * (kc + 1), :])
                nc.vector.tensor_sub(xt[kc][:], xt[kc][:], pt[:])

            # ================= round 1 =================
            qk1 = proj_qk(wqk1, xt, bqk_sb[1], 1)
            va1 = proj_v(wv1, xt, 1)
            attend(qk1, va1, bv_sb[1], corr_part)

            nc.gpsimd.collective_compute(
                "AllGather", mybir.AluOpType.bypass, replica_groups=GROUPS,
                ins=[corr_part[:, :]], outs=[corr_full[:, :]])

        # ================= gate + output ==================
        with ExitStack() as gat:
            gs_pool = gat.enter_context(tc.tile_pool(name="gs", bufs=16))
            gp_pool = gat.enter_context(tc.tile_pool(name="gp", bufs=8))
            wo_pool = gat.enter_context(tc.tile_pool(name="wo", bufs=KC))
            y_pool = gat.enter_context(tc.tile_pool(name="yp", bufs=2))
            ps2 = gat.enter_context(tc.tile_pool(name="ps2", bufs=3,
                                                 space="PSUM"))

            # Wg reuses the 8KB big8 slots freed by x^T/residual:
            # tile i holds contraction chunks 2i (cols 0:1024) and 2i+1.
            wg_t = []
            for i in range(KC):
                t_ = big8.tile([128, 2 * D], F32R, name=f"wg{i}", tag="b8")
                nc.sync.dma_start(t_[:, 0:D], wg[256 * i:256 * i + 128, :])
                nc.sync.dma_start(t_[:, D:2 * D],
                                  wg[256 * i + 128:256 * i + 256, :])
                wg_t.append(t_)

            def wg_slice(cc, jc):
                return wg_t[cc // 2][:, D * (cc % 2) + 128 * jc:
                                     D * (cc % 2) + 128 * (jc + 1)]

            wo_t = []
            for cc in range(KC):
                t_ = wo_pool.tile([128, D], F32R, name=f"wo{cc}", tag="wo")
                nc.sync.dma_start(t_[:], wo[128 * cc:128 * (cc + 1), :])
                wo_t.append(t_)

            pid = nc.sync.partition_id()
            greg = nc.sync.alloc_register("gslice")
            nc.sync.reg_mod(greg, pid, G)
            gsel = nc.sync.snap(greg, donate=True, min_val=0, max_val=G - 1)
            pred_v = pred_full[:, :].rearrange("d (s t) -> d s t", s=G)
            corr_v = corr_full[:, :].rearrange("d (s t) -> d s t", s=G)
            predg, corrg = [], []
            for cc in range(KC):
                pg_ = gs_pool.tile([128, TS], F32R, name=f"predg{cc}", tag="gs")
                cg_ = gs_pool.tile([128, TS], F32R, name=f"corrg{cc}", tag="gs")
                nc.sync.dma_start(
                    pg_[:], pred_v[128 * cc:128 * (cc + 1),
                                   bass.ds(gsel, 1), :].squeeze(1))
                nc.sync.dma_start(
                    cg_[:], corr_v[128 * cc:128 * (cc + 1),
                                   bass.ds(gsel, 1), :].squeeze(1))
                predg.append(pg_)
                corrg.append(cg_)

            pgt = []
            for jc in range(KC):
                ps = ps2.tile([128, 512], F32, tag="ps2", name="ps_g")
                for cc in range(16):
                    src = predg[cc] if cc < KC else corrg[cc - KC]
                    nc.tensor.matmul(ps[:], wg_slice(cc, jc), src[:],
                                     start=(cc == 0), stop=(cc == 15))
                gt = gp_pool.tile([128, TS], F32R, name=f"gate{jc}", tag="gp")
                nc.scalar.activation(gt[:], ps[:], SIG, bias=bg_sb[:, jc:jc + 1])
                nc.vector.tensor_mul(gt[:], gt[:], corrg[jc][:])
                nc.vector.tensor_add(gt[:], gt[:], predg[jc][:])
                pgt.append(gt)

            for tb in range(4):
                yt = y_pool.tile([128, D], F32, tag="y", name="yt")
                for n2 in range(2):
                    ps = ps2.tile([128, 512], F32, tag="ps2", name="ps_y")
                    for cc in range(KC):
                        nc.tensor.matmul(
                            ps[:], pgt[cc][:, 128 * tb:128 * (tb + 1)],
                            wo_t[cc][:, 512 * n2:512 * (n2 + 1)],
                            start=(cc == 0), stop=(cc == KC - 1))
                    nc.scalar.copy(yt[:, 512 * n2:512 * (n2 + 1)], ps[:])
                nc.sync.dma_start(y[128 * tb:128 * (tb + 1), :], yt[:])

    nc.compile()
    return nc


_NC = None


def _get_nc():
    global _NC
    if _NC is None:
        _NC = _build()
    return _NC


def make_in_maps(x, Wqkv0, bqkv0, Wqkv1, bqkv1, Wg, bg, Wo, bo):
    mask_np = np.where(np.arange(128)[:, None] > np.arange(128)[None, :],
                       np.float32(MASK_VAL), np.float32(0.0)).astype(np.float32)
    ones_np = np.ones((128, HG), np.float32)
    bg_a = np.ascontiguousarray(bg.reshape(D // 128, 128).T.astype(np.float32))
    wg_np = np.ascontiguousarray(Wg.astype(np.float32))
    wo_np = np.ascontiguousarray(Wo.astype(np.float32))

    in_maps = []
    for c in range(8):
        b, g = divmod(c, G)
        cq = slice(CP * g, CP * (g + 1))
        ck = slice(D + CP * g, D + CP * (g + 1))
        cv = slice(2 * D + CP * g, 2 * D + CP * (g + 1))
        m = {
            "xT": np.ascontiguousarray(x[b].T.astype(np.float32)),
            "mask": mask_np, "onesc": ones_np, "bg": bg_a,
            "wg": wg_np, "wo": wo_np,
        }
        for r, (W, bb) in enumerate(((Wqkv0, bqkv0), (Wqkv1, bqkv1))):
            m[f"wqk{r}"] = np.ascontiguousarray(
                np.concatenate([W[:, cq], W[:, ck]], axis=1).astype(np.float32))
            m[f"wv{r}"] = np.ascontiguousarray(W[:, cv].astype(np.float32))
            bqk_cat = np.concatenate([bb[cq], bb[ck]]).astype(np.float32)
            m[f"bqk{r}"] = np.ascontiguousarray(bqk_cat.reshape(4, 128).T)
            m[f"bv{r}"] = np.ascontiguousarray(
                bb[cv].astype(np.float32).reshape(HG, 64).T)
        in_maps.append(m)
    return in_maps


def assemble(results, bo):
    out = np.empty((B, T, D), np.float32)
    for c in range(8):
        b, g = divmod(c, G)
        out[b, TS * g:TS * (g + 1), :] = results[c]["y"]
    return out + bo.astype(np.float32)


def kernel(x, Wqkv0, bqkv0, Wqkv1, bqkv1, Wg, bg, Wo, bo):
    nc = _get_nc()
    in_maps = make_in_maps(x, Wqkv0, bqkv0, Wqkv1, bqkv1, Wg, bg, Wo, bo)
    res = bass_utils.run_bass_kernel_spmd(nc, in_maps, core_ids=list(range(8)))
    return assemble(res.results, bo)
